# revision 28
# baseline (speedup 1.0000x reference)
# Trainium2 Bass kernel for nn_AttentiveLinear.
#
# Math:  y[n,o] = sum_i x[n,i] * W[n,i,o] + b[n,o]
#        W[n,i,o] = (x @ Ww)[n, i*128+o] + bw[i*128+o]
#        b        = x @ Wb + bb
# Expanded:
#        y[n,o] = sum_i x[n,i] * T[n,i,o]  +  (x @ (Wb + BW))[n,o] + bb[o]
# with   T = x @ Ww (the 512MB intermediate, kept on-chip only) and
#        BW[i,o] = bw[i*128+o].
#
# Per-core plan (data-parallel over tokens, 1024 tokens/core):
#   pass 1: for each output o (=chunk c), matmul
#           Tc[i, tok] = Wq_c^T @ xT   (Wq_c = Ww columns for o=c, [j, i])
#           PSUM -> SBUF copies (DVE+ACT alternating) store T as bf16 in
#           token-major layout tb[i, tok, o] so each token's T_n[i, o] is a
#           contiguous 128x128 stationary operand.
#   pass 2: yT_psum[o, tok]  = lin^T @ xT  (linear part, one matmul)
#                            += per-token matmul T_n^T @ x_n (M=128 dense)
#           bias added during the PSUM->SBUF copy via per-partition scalar add.
# Host does all layout prep: x transpose/shard/cast, Ww column permutation,
# folding bw into the linear weight.

import numpy as np
import ml_dtypes

N_CORES = 8
IN_F = 128
OUT_F = 128
TOK_TOTAL = 8192
TOK = TOK_TOTAL // N_CORES  # 1024 tokens per core
# Token groups per core. Uneven: the last (small) group keeps the final
# un-interleaved pass-2 drain short.
GROUPS = [256, 256, 256, 256]
assert sum(GROUPS) == TOK
G = max(GROUPS)
GP = 256  # PSUM chunk-slot stride (keeps each chunk inside one bank)

_CACHE = {}
LAST_RESULT = None


def _build_program():
    import concourse.mybir as mybir
    import concourse.tile as tile
    from concourse import bacc

    from concourse.tile_rust import add_dep_helper

    dt = mybir.dt
    nc = bacc.Bacc(
        "TRN2", target_bir_lowering=False, debug=False, num_devices=N_CORES
    )

    xt_d = nc.dram_tensor("xt", [IN_F, TOK], dt.bfloat16, kind="ExternalInput")
    wq_d = nc.dram_tensor(
        "wq", [IN_F, IN_F * OUT_F], dt.bfloat16, kind="ExternalInput"
    )
    lin_d = nc.dram_tensor("lin", [IN_F, OUT_F], dt.bfloat16, kind="ExternalInput")
    bbc_d = nc.dram_tensor("bbc", [OUT_F, 1], dt.float32, kind="ExternalInput")
    yt_d = nc.dram_tensor("yt", [OUT_F, TOK], dt.float32, kind="ExternalOutput")

    with tile.TileContext(nc) as tc:
        with (
            tc.tile_pool(name="const", bufs=1) as const,
            tc.tile_pool(name="tbig", bufs=2) as tbigp,
            tc.tile_pool(name="ysb", bufs=2) as ysbp,
            tc.tile_pool(name="psch", bufs=3, space="PSUM") as psch,
            tc.tile_pool(name="psy", bufs=2, space="PSUM") as psyp,
        ):
            lin_s = const.tile([IN_F, OUT_F], dt.bfloat16)
            nc.sync.dma_start(lin_s[:], lin_d[:])
            bbc_s = const.tile([OUT_F, 1], dt.float32)
            nc.sync.dma_start(bbc_s[:], bbc_d[:])
            xt_s = const.tile([IN_F, TOK], dt.bfloat16)
            OFFS = [sum(GROUPS[:i]) for i in range(len(GROUPS) + 1)]
            for g, sz in enumerate(GROUPS):
                nc.gpsimd.dma_start(
                    xt_s[:, OFFS[g] : OFFS[g + 1]], xt_d[:, OFFS[g] : OFFS[g + 1]]
                )
            wq_s = const.tile([IN_F, IN_F * OUT_F], dt.bfloat16)
            dma_engines = [nc.sync, nc.gpsimd, nc.scalar]
            for k in range(32):
                sl = slice(k * 512, (k + 1) * 512)
                dma_engines[k % 3].dma_start(wq_s[:, sl], wq_d[:, sl])

            NQ = OUT_F // 4  # 32 quad-chunks per group

            def emit_pass2_tokens(g, yp, tb, t0, t1, after=None):
                # per-token matmuls accumulating y^T columns for group g
                sz = GROUPS[g]
                last = None
                for t in range(t0, t1):
                    n = OFFS[g] + t
                    last = nc.tensor.matmul(
                        yp[:, t : t + 1],
                        tb[:, t, :],
                        xt_s[:, n : n + 1],
                        start=False,
                        stop=(t == sz - 1),
                        skip_group_check=True,
                    )
                    if after is not None:
                        add_dep_helper(
                            last.ins,
                            after.ins,
                            sync=False,
                            reason="pass-2 batch after this quad's chunks",
                        )
                        after = None
                return last

            def finish_group(g, yp):
                sz = GROUPS[g]
                ys = ysbp.tile([OUT_F, G], dt.float32)
                nc.vector.tensor_scalar_add(ys[:, :sz], yp[:, :sz], bbc_s[:])
                nc.sync.dma_start(yt_d[:, OFFS[g] : OFFS[g + 1]], ys[:, :sz])

            # HAM warmup: run dummy matmuls on the first xt slice as soon as
            # it lands so the PE reaches the warm (2.4 GHz) clock right as
            # pass 1 starts (wq still streaming in).
            wps = psch.tile([IN_F, 4, GP], dt.float32, tag="ps")
            for w in range(12):
                nc.tensor.matmul(
                    wps[:, w % 4, 0:G],
                    xt_s[:, 0:IN_F],
                    xt_s[:, 0:G],
                    start=True,
                    stop=True,
                )

            prev = None  # (g, yp, tb) of previous group awaiting pass-2
            # Interleave the previous group's pass-2 starting at quad 4 so
            # the PE has chunk work queued while that group's final copies
            # drain (every token-matmul needs all 32 copies done).
            START_Q = 4
            last_tok = None  # forces the scheduler to keep the interleave
            for g, sz in enumerate(GROUPS):
                gs = slice(OFFS[g], OFFS[g + 1])
                # tb[i, tok_in_group, o], bf16
                tb = tbigp.tile([IN_F, G, OUT_F], dt.bfloat16)
                bounds = (
                    None
                    if prev is None
                    else np.linspace(0, GROUPS[prev[0]], NQ - START_Q + 1).astype(int)
                )

                # pass 1 for group g, with the previous group's pass-2
                # token-matmuls interleaved to keep the PE array warm.
                for cq in range(NQ):
                    ps = psch.tile([IN_F, 4, GP], dt.float32, tag="ps")
                    last_chunk = None
                    for q in range(4):
                        c = cq * 4 + q
                        last_chunk = nc.tensor.matmul(
                            ps[:, q, 0:sz],
                            wq_s[:, c * IN_F : (c + 1) * IN_F],
                            xt_s[:, gs],
                            start=True,
                            stop=True,
                        )
                        if q == 0 and last_tok is not None:
                            add_dep_helper(
                                last_chunk.ins,
                                last_tok.ins,
                                sync=False,
                                reason="keep pass-2 interleaved with pass-1",
                            )
                            last_tok = None
                    # transposing copy: strided PSUM read, blocked SBUF write
                    in_ap = ps[:, :, 0:sz].transpose([0, 2, 1])  # [128, sz, 4]
                    out_ap = tb[:, 0:sz, cq * 4 : (cq + 1) * 4]  # [128, sz, 4]
                    if cq % 2 == 0:
                        nc.vector.tensor_copy(out_ap, in_ap)
                    else:
                        nc.scalar.copy(out_ap, in_ap)
                    if prev is not None and cq >= START_Q:
                        last_tok = emit_pass2_tokens(
                            prev[0],
                            prev[1],
                            prev[2],
                            int(bounds[cq - START_Q]),
                            int(bounds[cq - START_Q + 1]),
                            after=last_chunk,
                        )
                if prev is not None:
                    finish_group(prev[0], prev[1])

                # init this group's y^T PSUM bank with the linear part
                yp = psyp.tile([OUT_F, G], dt.float32)
                nc.tensor.matmul(
                    yp[:, 0:sz],
                    lin_s[:],
                    xt_s[:, gs],
                    start=True,
                    stop=False,
                    skip_group_check=True,
                )
                prev = (g, yp, tb)

            # drain the last group's pass-2
            emit_pass2_tokens(prev[0], prev[1], prev[2], 0, GROUPS[prev[0]])
            finish_group(prev[0], prev[1])

    nc.compile()
    return nc


def _host_prep(x, Wb, bb, Ww, bw):
    bf16 = ml_dtypes.bfloat16
    x = np.asarray(x, dtype=np.float32)
    Wb = np.asarray(Wb, dtype=np.float32)
    bb = np.asarray(bb, dtype=np.float32)
    Ww = np.asarray(Ww, dtype=np.float32)
    bw = np.asarray(bw, dtype=np.float32)

    xf = x.reshape(-1, IN_F)
    # Wq[j, o*128 + i] = Ww[j, i*128 + o]
    wq = np.ascontiguousarray(
        Ww.reshape(IN_F, IN_F, OUT_F).transpose(0, 2, 1)
    ).reshape(IN_F, IN_F * OUT_F).astype(bf16)
    lin = (Wb + bw.reshape(IN_F, OUT_F)).astype(bf16)
    bbc = np.ascontiguousarray(bb.reshape(OUT_F, 1))

    in_maps = []
    for c in range(N_CORES):
        sh = xf[c * TOK : (c + 1) * TOK]
        xt = np.ascontiguousarray(sh.T).astype(bf16)
        in_maps.append({"xt": xt, "wq": wq, "lin": lin, "bbc": bbc})
    return in_maps, x.shape


def _ensure_trace_support():
    """If profiling is requested (BASS_TRACE) on an image without
    antenv.axon_hooks, synthesize the hook module so tracing works instead
    of crashing, and keep artifact upload local (no bucket access)."""
    import sys
    import types

    try:
        import antenv

        try:
            from antenv.axon_hooks import get_axon_ntff_profile_hook  # noqa: F401
        except ImportError:
            hook = None
            try:
                from trn_agent_boot.trn_boot import _ntff_profile_via_ctypes

                hook = _ntff_profile_via_ctypes("/opt/axon/libaxon_pjrt.so")
            except Exception:
                pass
            m = types.ModuleType("antenv.axon_hooks")
            hooks = {"h": hook}
            m.get_axon_ntff_profile_hook = lambda: hooks["h"]
            m.set_axon_ntff_profile_hook = lambda h: hooks.__setitem__("h", h)
            sys.modules["antenv.axon_hooks"] = m
            antenv.axon_hooks = m
    except Exception:
        pass
    try:
        import concourse.bass_utils as bu
        from concourse._compat import FishPath

        FishPath.bucket_root()
    except Exception:
        try:
            bu.upload_artifacts = lambda tmpdir: tmpdir
        except Exception:
            pass


def kernel(x, Wb, bb, Ww, bw):
    global LAST_RESULT
    _ensure_trace_support()
    from concourse.bass_utils import run_bass_kernel_spmd

    in_maps, xshape = _host_prep(x, Wb, bb, Ww, bw)
    if "nc" not in _CACHE:
        _CACHE["nc"] = _build_program()
    nc = _CACHE["nc"]

    res = run_bass_kernel_spmd(nc, in_maps, core_ids=list(range(N_CORES)))
    LAST_RESULT = res
    y = np.concatenate(
        [res.results[c]["yt"].T for c in range(N_CORES)], axis=0
    )
    return np.ascontiguousarray(y.reshape(xshape[:-1] + (OUT_F,)), dtype=np.float32)
